# revision 33
# baseline (speedup 1.0000x reference)
"""BirthDeathAttention kernel for 8 Trainium2 NeuronCores.

Math note: in the reference, both `persistence_bias` ([1,H,1,1]) and
`importance_weights[:, None, :, None] * 0.1` ([B,1,N,1]) are constant along
the softmax (key) axis, so they cancel exactly inside the softmax.  The
module is therefore plain multi-head attention + output projection.

Sharding (per the tensor-parallel hint): core = (batch b, head-group g),
b in {0,1}, g in {0..3}, each core handling 4 of the 16 heads for one batch
element.  Each core computes a partial output projection (its heads' slice
of W_proj rows); the host sums the 4 partials per batch and adds b_proj.

Schedule: wave-pipelined.  Wave w = (block b = w//16, key-tile c = w%16):
  S-pair(w):  two row-tiled concurrent matmuls (K=64 per head, the pair's
              heads at PE rows 0-63 / 64-127) -> one [128,1024] PSUM tile
  exp(w):     one ACTIVATE [128,1024] PSUM->SBUF bf16 (~1.15us, the pacer
              once the PE stops being the constraint)
  U(w):       two serial matmuls (M=65: v|ones so the softmax denominator
              lands in row 64) accumulating into the block's U PSUM pair
On hardware one S-pair plus two U matmuls cost ~1.09us of PE issue time —
barely under the 1.15us exp — so the q/k/v projection chains cannot hide
inside U-active cycles.  Instead U is *delayed*: all chains drain from a
deadline-checked FIFO during waves 0..~45 (early S-blocks run PE-bound,
exp rides along), then U catches up at 2 waves/cycle on four alternating
PSUM banks (u_cycle(w) = max(w+3, 46+w//2)) until it trails exp by 3
cycles.  The output projection runs entirely in the tail on the freed psS
banks, with pair-0 matmuls flowing while the last block normalizes.

PSUM (8 banks): psS 2x[128,1024] (4) + psU slots s0..s3 (4).  U blocks
alternate (s0,s1)/(s2,s3); chain accumulators borrow the same slots via
tag rotation before U claims them.
"""

import sys

if "/opt/trn_rl_repo" not in sys.path:
    sys.path.insert(0, "/opt/trn_rl_repo")

from collections import deque

import numpy as np
import ml_dtypes

import concourse.bass as bass
import concourse.mybir as mybir
import concourse.tile as tile
from concourse.bass_utils import run_bass_kernel_spmd

DIM = 1024
N = 2048
B = 2
HEADS = 16
HEAD_DIM = 64
SCALE = HEAD_DIM ** -0.5
HPG = 4          # heads per group (per core)
GC = HPG * HEAD_DIM  # channels per core = 256
BF16 = mybir.dt.bfloat16
F32 = mybir.dt.float32

KT = DIM // 128   # 8 contraction tiles over model dim
CT_COL = {0: 1, 1: 3, 2: 0, 3: 2}  # chain ct -> wqk column block
NKT = N // 128    # 16 key tiles per block
NW = 8 * NKT      # 128 waves
# block order: (nqb, pair) — interleaved so pair-1 q/k chains have relaxed
# deadlines
BLOCKS = [(0, 0), (1, 0), (2, 0), (3, 0), (0, 1), (1, 1), (2, 1), (3, 1)]
U_START = 44      # first cycle with U matmuls (chains fully drain first:
                  # chain PSUM accumulators borrow the U slots, so any
                  # overlap of the two lifetimes deadlocks the in-order PE)


def _u_cycle(w):
    """U emission cycle for wave w: 1.5-waves/cycle catch-up from U_START
    (3 matmuls/cycle keeps the PE under the ~1.15us exp pace, unlike
    2/cycle which made the PE the pacer) until U trails exp by 3 cycles.
    ~1.67 waves/cycle clears the backlog by cycle ~110 so the last
    two blocks are not congested."""
    return max(w + 3, U_START + (3 * w) // 5)


def _split_multi_waits(nc, max_waits=1):
    """The walrus build in this container accepts at most one sync-wait per
    instruction.  Hoist extra waits onto single-wait NOPs inserted just
    before the instruction in its engine's program order."""
    uid = [0]
    for f in nc.m.functions:
        for bb in f.blocks:
            insts = bb.instructions
            new = []
            changed = False
            for inst in insts:
                si = inst.sync_info
                if si is not None and len(si.on_wait) > max_waits:
                    waits = list(si.on_wait)
                    for w in waits[:-max_waits]:
                        nop = mybir.InstNoOp(
                            name=f"I-splitw-{uid[0]}", ins=[], outs=[])
                        uid[0] += 1
                        nop.engine = inst.engine
                        nop.sync_info = mybir.SyncInfo(
                            on_wait=[w], on_update=[])
                        new.append(nop)
                    si.on_wait = waits[-max_waits:]
                    inst.sync_info = si
                    changed = True
                new.append(inst)
            if changed:
                bb.instructions = new


def build_core_kernel() -> bass.Bass:
    nc = bass.Bass()
    xT = nc.declare_dram_parameter("xT", [DIM, N], BF16, isOutput=False)
    wqk = nc.declare_dram_parameter("wqk", [DIM, 2 * GC], BF16, isOutput=False)
    wv = nc.declare_dram_parameter("wv", [DIM, GC], BF16, isOutput=False)
    wp = nc.declare_dram_parameter("wp", [GC, DIM], BF16, isOutput=False)
    out = nc.declare_dram_parameter("out", [N, DIM], BF16, isOutput=True)

    xT_r = xT.rearrange("(kt p) n -> p kt n", p=128)
    wqk_r = wqk.rearrange("(kt p) c -> p kt c", p=128)
    wv_r = wv.rearrange("(kt p) c -> p kt c", p=128)
    wp_r = wp.rearrange("(pair p) c -> p pair c", p=128)

    with tile.TileContext(nc) as tc:
        from contextlib import ExitStack

        with ExitStack() as ctx:
            consts = ctx.enter_context(tc.tile_pool(name="consts", bufs=1))
            sbuf = ctx.enter_context(tc.tile_pool(name="sbuf", bufs=1))
            epool = ctx.enter_context(tc.tile_pool(name="epool", bufs=46))
            npool = ctx.enter_context(tc.tile_pool(name="npool", bufs=2))
            rdram = ctx.enter_context(
                tc.tile_pool(name="rdram", bufs=2, space="DRAM"))
            opool = ctx.enter_context(tc.tile_pool(name="opool", bufs=4))
            psS = ctx.enter_context(
                tc.tile_pool(name="psS", bufs=2, space="PSUM"))
            psU = ctx.enter_context(
                tc.tile_pool(name="psU", bufs=1, space="PSUM"))

            # --- resident SBUF tensors -------------------------------------
            xT_sb = sbuf.tile([128, KT, N], BF16, tag="xT")
            wqk_sb = consts.tile([128, KT, 2 * GC], BF16, tag="wqk")
            wv_sb = consts.tile([128, KT, GC], BF16, tag="wv")
            wp_sb = consts.tile([128, 2, DIM], BF16, tag="wp")
            ones_sb = consts.tile([128, 1], BF16, tag="ones")
            warm_sb = consts.tile([128, 512], BF16, tag="warm")
            qk_sb = sbuf.tile([128, 4, N], BF16, tag="qk")
            # v with a ones column appended per head ([v_h | 1], stride 65):
            # the ones column turns attention@v into a matmul that also
            # emits the softmax denominator as output row 64
            v_sb = sbuf.tile([128, NKT, HPG * 65], BF16, tag="v")
            o_sb = sbuf.tile([128, 2, N], BF16, tag="o")

            nc.vector.memset(ones_sb[:], 1.0)
            nc.vector.memset(warm_sb[:], 1.0)
            v_view = v_sb.rearrange("p nt (h c) -> p nt h c", c=65)
            nc.vector.memset(v_view[:, :, :, 64:65], 1.0)

            # --- input DMAs: whole tensors (contiguous >=512B per-partition
            # segments).  wqk first on the scalar queue: the first chains
            # need it.
            for kt in range(KT):
                nc.scalar.dma_start(out=wqk_sb[:, kt, 0:256],
                                    in_=wqk_r[:, kt, 0:256])
            nc.scalar.dma_start(out=wqk_sb[:, :, 256:512],
                                in_=wqk_r[:, :, 256:512])
            nc.scalar.dma_start(out=wv_sb[:], in_=wv_r[:])
            nc.scalar.dma_start(out=wp_sb[:], in_=wp_r[:])
            for nb in range(4):
                nc.sync.dma_start(out=xT_sb[:, :, nb * 512:(nb + 1) * 512],
                                  in_=xT_r[:, :, nb * 512:(nb + 1) * 512])

            # --- wave emitters --------------------------------------------
            def s_pair(w):
                nqb, pair = BLOCKS[w // NKT]
                nkt = w % NKT
                qt = qk_sb[:, pair, :]
                kt_sb = qk_sb[:, 2 + pair, :]
                st = psS.tile([128, 1024], F32, tag="st")
                for hh in range(2):
                    nc.tensor.matmul(
                        st[:, hh * 512:(hh + 1) * 512],
                        lhsT=kt_sb[hh * 64:(hh + 1) * 64,
                                   nkt * 128:(nkt + 1) * 128],
                        rhs=qt[hh * 64:(hh + 1) * 64,
                               nqb * 512:(nqb + 1) * 512],
                        start=True,
                        stop=True,
                    )
                return st

            def exp_wave(st):
                e_t = epool.tile([128, 1024], BF16, tag="e")
                nc.scalar.activation(
                    e_t[:], st[:],
                    mybir.ActivationFunctionType.Exp,
                    scale=SCALE,
                )
                return e_t

            def u_wave(u_pair, w, e_t):
                pair = BLOCKS[w // NKT][1]
                nkt = w % NKT
                for hh, u_t in ((0, u_pair[0]), (1, u_pair[1])):
                    h = pair * 2 + hh
                    nc.tensor.matmul(
                        u_t[0:65, :],
                        lhsT=v_sb[:, nkt, h * 65:h * 65 + 65],
                        rhs=e_t[:, hh * 512:(hh + 1) * 512],
                        start=(nkt == 0),
                        stop=(nkt == NKT - 1),
                    )

            def norm_block(b, u_pair):
                """Normalize a block's U pair into o_sb.  Blocks 0-5 first
                copy U out of PSUM (bank turnover); 6-7 go straight from
                PSUM for a shorter tail."""
                nqb, pair = BLOCKS[b]
                u_a, u_b = u_pair
                late = b >= 6
                if not late:
                    uc_a = npool.tile([65, 512], F32, tag="uc_a")
                    uc_b = npool.tile([65, 512], F32, tag="uc_b")
                    nc.vector.tensor_copy(uc_a[:], u_a[0:65, :])
                    nc.vector.tensor_copy(uc_b[:], u_b[0:65, :])
                    u_a, u_b = uc_a, uc_b
                # both denominator rows into one tile so one reciprocal
                # covers them (partition offsets must be 32-aligned and a
                # span from offset 32 may not exceed 32 partitions).  The
                # first copy fills rows 0-31 with harmless v-values so
                # every reciprocal input byte is initialized.
                dn = npool.tile([33, 512], F32, tag="dn", bufs=1)
                rec = npool.tile([33, 512], F32, tag="rec", bufs=1)
                nc.vector.tensor_copy(dn[0:32, :], u_a[32:64, :])
                nc.vector.tensor_copy(dn[32:33, :], u_a[64:65, :])
                nc.vector.tensor_copy(dn[0:1, :], u_b[64:65, :])
                nc.vector.reciprocal(rec[:], dn[:])
                rr_a = npool.tile([64, 512], F32, tag="rr_a", bufs=1)
                rr_b = npool.tile([64, 512], F32, tag="rr_b", bufs=1)
                r_dr = rdram.tile([2, 512], F32, tag="rdr")
                nc.sync.dma_start(out=r_dr[0:1, :], in_=rec[32:33, :])
                nc.sync.dma_start(out=r_dr[1:2, :], in_=rec[0:1, :])
                nc.sync.dma_start(
                    out=rr_a[:], in_=r_dr[0:1, :].to_broadcast([64, 512]))
                nc.sync.dma_start(
                    out=rr_b[:], in_=r_dr[1:2, :].to_broadcast([64, 512]))
                nc.vector.tensor_mul(
                    o_sb[0:64, pair, nqb * 512:(nqb + 1) * 512],
                    u_a[0:64, :], rr_a[:],
                )
                nc.vector.tensor_mul(
                    o_sb[64:128, pair, nqb * 512:(nqb + 1) * 512],
                    u_b[0:64, :], rr_b[:],
                )

            # --- chain FIFO -----------------------------------------------
            # Items: (deadline_cycle, pe_cost_ns, thunk).  Drained in order
            # while the PE budget lasts; anything overdue force-drains.
            fifo = deque()
            MM_NS = 215
            slot_rot = [0]

            def u_slot_tile(name):
                t = psU.tile([128, 512], F32, tag=f"s{slot_rot[0] % 4}",
                             name=name)
                slot_rot[0] += 1
                return t

            def q_chain(kind, ct_or_nt, nb, deadline, slot=None):
                state = {}

                def first_mm():
                    if slot is None:
                        state["acc"] = u_slot_tile("acc")
                    else:
                        state["acc"] = psU.tile([128, 512], F32,
                                                tag=f"s{slot}", name="acc")
                    chain_mm(0)

                def chain_mm(kt):
                    acc = state["acc"]
                    if kind == "a":
                        nc.tensor.matmul(
                            acc[:],
                            lhsT=wqk_sb[:, kt, CT_COL[ct_or_nt] * 128:
                                        CT_COL[ct_or_nt] * 128 + 128],
                            rhs=xT_sb[:, kt, nb * 512:(nb + 1) * 512],
                            start=(kt == 0), stop=(kt == KT - 1),
                        )
                    else:
                        nc.tensor.matmul(
                            acc[:, 0:GC],
                            lhsT=xT_sb[:, kt,
                                       ct_or_nt * 128:(ct_or_nt + 1) * 128],
                            rhs=wv_sb[:, kt, :],
                            start=(kt == 0), stop=(kt == KT - 1),
                        )

                def copy_out():
                    acc = state["acc"]
                    if kind == "a":
                        nc.vector.tensor_copy(
                            qk_sb[:, ct_or_nt, nb * 512:(nb + 1) * 512],
                            acc[:],
                        )
                    else:
                        nc.vector.tensor_copy(
                            v_view[:, ct_or_nt, :, 0:64],
                            acc[:, 0:GC].rearrange("p (h c) -> p h c", c=64),
                        )

                fifo.append((deadline, MM_NS, first_mm))
                for kt in range(1, KT):
                    fifo.append((deadline, MM_NS,
                                 lambda kt=kt: chain_mm(kt)))
                fifo.append((deadline, 0, copy_out))

            # chain order/deadlines: the copy must be EMITTED before the
            # first wave that consumes it (the Tile dependency tracker
            # captures deps in emission order)
            q_chain("a", 2, 2, 6)
            q_chain("a", 2, 3, 10)
            q_chain("a", 0, 1, 14)
            for nt in range(3):
                q_chain("b", nt, 0, 26 + nt)
            q_chain("a", 0, 2, 30)
            for nt in range(3, 13):
                q_chain("b", nt, 0, min(30 + nt, 42))
            q_chain("a", 0, 3, 36)
            q_chain("a", 3, 0, 37)
            q_chain("a", 1, 0, 38)
            q_chain("a", 3, 1, 39)
            q_chain("a", 3, 2, 40)
            q_chain("a", 3, 3, 41)
            q_chain("a", 1, 1, 42)
            q_chain("a", 1, 2, 43)
            q_chain("a", 1, 3, 43)
            # the last three v-chains may spill past U_START; pin them to
            # the s2/s3 slots, which U only claims at block 1 (cycle ~52)
            q_chain("b", 13, 0, 44, slot=2)
            q_chain("b", 14, 0, 45, slot=3)
            q_chain("b", 15, 0, 46, slot=2)

            # U emission map: catch-up at 2 waves/cycle after U_START until
            # trailing exp by 3 cycles
            u_emit = {}
            for w in range(NW):
                u_emit.setdefault(_u_cycle(w), []).append(w)

            # --- prologue --------------------------------------------------
            # HAM warm-up: harmless M=1 matmuls bridge the DMA wait so the
            # PE clock is at 8/8 when the first real chains run
            warm_ps = u_slot_tile("warm_ps")
            for i in range(12):
                nc.tensor.matmul(
                    warm_ps[0:1, :],
                    lhsT=ones_sb[:, 0:1], rhs=warm_sb[:],
                    start=(i == 0), stop=(i == 11),
                )
            for ct, nb in ((2, 0), (0, 0), (2, 1)):
                accp = u_slot_tile(f"accp{ct}{nb}")
                for kt in range(KT):
                    nc.tensor.matmul(
                        accp[:],
                        lhsT=wqk_sb[:, kt, CT_COL[ct] * 128:
                                    CT_COL[ct] * 128 + 128],
                        rhs=xT_sb[:, kt, nb * 512:(nb + 1) * 512],
                        start=(kt == 0), stop=(kt == KT - 1))
                nc.vector.tensor_copy(
                    qk_sb[:, ct, nb * 512:(nb + 1) * 512], accp[:])

            u_pairs = {}

            def get_u_pair(b):
                if b not in u_pairs:
                    base = (b % 2) * 2
                    t_a = psU.tile([128, 512], F32, tag=f"s{base}",
                                   name=f"ua{b}")
                    t_b = psU.tile([128, 512], F32, tag=f"s{base + 1}",
                                   name=f"ub{b}")
                    u_pairs[b] = (t_a, t_b)
                return u_pairs[b]

            # --- main loop -------------------------------------------------
            e_tiles = {}
            st_prev = s_pair(0)
            for cyc in range(NW + 6):
                budget = 1100.0
                if cyc < NW:
                    e_tiles[cyc] = exp_wave(st_prev)
                    if cyc + 1 < NW:
                        st_prev = s_pair(cyc + 1)
                        budget -= 216
                for w in u_emit.get(cyc, []):
                    b = w // NKT
                    u_wave(get_u_pair(b), w, e_tiles[w])
                    budget -= 2 * MM_NS
                    if w % NKT == NKT - 1:
                        norm_block(b, u_pairs[b])
                spent = 0.0
                while fifo and (
                    any(d is not None and d <= cyc for d, _, _ in fifo)
                    or spent + fifo[0][1] <= budget
                ):
                    _, cost, thunk = fifo.popleft()
                    thunk()
                    spent += cost
            while fifo:
                fifo.popleft()[2]()

            # --- tail: full output projection on the freed psS banks ------
            # pair-0 matmuls of the last quad flow while block 7 normalizes
            def proj_piece(mt):
                ot = opool.tile([128, DIM], BF16, tag="ot", name="ot")
                for nh in range(2):
                    acc_w = psS.tile([128, 1024], F32, tag="st",
                                     name="tailacc")
                    acc = acc_w[:, 0:512]
                    for pair in range(2):
                        nc.tensor.matmul(
                            acc[:],
                            lhsT=o_sb[:, pair, mt * 128:(mt + 1) * 128],
                            rhs=wp_sb[:, pair, nh * 512:(nh + 1) * 512],
                            start=(pair == 0), stop=(pair == 1),
                        )
                    nc.scalar.copy(ot[:, nh * 512:(nh + 1) * 512], acc[:])
                eng = nc.scalar if mt % 2 else nc.sync
                eng.dma_start(out=out[mt * 128:(mt + 1) * 128, :], in_=ot[:])

            for mt in [0, 1, 2, 3, 4, 5, 6, 7, 8, 9, 10, 11, 12, 13,
                       14, 15]:
                proj_piece(mt)

    _split_multi_waits(nc)
    return nc


_NC_CACHE = None


def _get_nc():
    global _NC_CACHE
    if _NC_CACHE is None:
        _NC_CACHE = build_core_kernel()
    return _NC_CACHE


def kernel(x, importance_weights, W_qkv, W_proj, b_proj, persistence_bias,
           _results_hook=None):
    x = np.asarray(x)
    W_qkv = np.asarray(W_qkv, dtype=np.float32)
    W_proj = np.asarray(W_proj, dtype=np.float32)
    b_proj = np.asarray(b_proj, dtype=np.float32)

    bf = ml_dtypes.bfloat16
    Q = W_qkv[:, 0:DIM]
    K = W_qkv[:, DIM:2 * DIM]
    V = W_qkv[:, 2 * DIM:3 * DIM]

    in_maps = []
    for core in range(8):
        b, g = divmod(core, 4)
        sl = slice(g * GC, (g + 1) * GC)
        in_maps.append({
            "xT": np.ascontiguousarray(x[b].T).astype(bf),
            # [k-pair0 | q-pair0 | k-pair1 | q-pair1]: the first half
            # unlocks the first chains with a half-tensor DMA
            "wqk": np.ascontiguousarray(np.concatenate(
                [K[:, sl][:, 0:128], Q[:, sl][:, 0:128],
                 K[:, sl][:, 128:256], Q[:, sl][:, 128:256]],
                axis=1)).astype(bf),
            "wv": np.ascontiguousarray(V[:, sl]).astype(bf),
            "wp": np.ascontiguousarray(W_proj[sl, :]).astype(bf),
        })

    nc = _get_nc()
    res = run_bass_kernel_spmd(nc, in_maps, list(range(8)))
    if _results_hook is not None:
        _results_hook(res)

    out = np.zeros((B, N, DIM), dtype=np.float32)
    for core in range(8):
        b = core // 4
        out[b] += res.results[core]["out"].astype(np.float32)
    out += b_proj[None, None, :]
    return out


# revision 34
# speedup vs baseline: 1.0453x; 1.0453x over previous
"""BirthDeathAttention kernel for 8 Trainium2 NeuronCores.

Math note: in the reference, both `persistence_bias` ([1,H,1,1]) and
`importance_weights[:, None, :, None] * 0.1` ([B,1,N,1]) are constant along
the softmax (key) axis, so they cancel exactly inside the softmax.  The
module is therefore plain multi-head attention + output projection.

Sharding (per the tensor-parallel hint): core = (batch b, head-group g),
b in {0,1}, g in {0..3}, each core handling 4 of the 16 heads for one batch
element.  Each core computes a partial output projection (its heads' slice
of W_proj rows); the host sums the 4 partials per batch and adds b_proj.

Schedule: wave-pipelined.  Wave w = (block b = w//16, key-tile c = w%16):
  S-pair(w):  two row-tiled concurrent matmuls (K=64 per head, the pair's
              heads at PE rows 0-63 / 64-127) -> one [128,1024] PSUM tile
  exp(w):     one ACTIVATE [128,1024] PSUM->SBUF bf16 (~1.15us, the pacer
              once the PE stops being the constraint)
  U(w):       two serial matmuls (M=65: v|ones so the softmax denominator
              lands in row 64) accumulating into the block's U PSUM pair
On hardware one S-pair plus two U matmuls cost ~1.09us of PE issue time —
barely under the 1.15us exp — so the q/k/v projection chains cannot hide
inside U-active cycles.  Instead U is *delayed*: all chains drain from a
deadline-checked FIFO during waves 0..~45 (early S-blocks run PE-bound,
exp rides along), then U catches up at 2 waves/cycle on four alternating
PSUM banks (u_cycle(w) = max(w+3, 46+w//2)) until it trails exp by 3
cycles.  The output projection runs entirely in the tail on the freed psS
banks, with pair-0 matmuls flowing while the last block normalizes.

PSUM (8 banks): psS 2x[128,1024] (4) + psU slots s0..s3 (4).  U blocks
alternate (s0,s1)/(s2,s3); chain accumulators borrow the same slots via
tag rotation before U claims them.
"""

import sys

if "/opt/trn_rl_repo" not in sys.path:
    sys.path.insert(0, "/opt/trn_rl_repo")

from collections import deque

import numpy as np
import ml_dtypes

import concourse.bass as bass
import concourse.mybir as mybir
import concourse.tile as tile
from concourse.bass_utils import run_bass_kernel_spmd

DIM = 1024
N = 2048
B = 2
HEADS = 16
HEAD_DIM = 64
SCALE = HEAD_DIM ** -0.5
HPG = 4          # heads per group (per core)
GC = HPG * HEAD_DIM  # channels per core = 256
BF16 = mybir.dt.bfloat16
F32 = mybir.dt.float32

KT = DIM // 128   # 8 contraction tiles over model dim
CT_COL = {0: 1, 1: 3, 2: 0, 3: 2}  # chain ct -> wqk column block
NKT = N // 128    # 16 key tiles per block
NW = 8 * NKT      # 128 waves
# block order: (nqb, pair) — interleaved so pair-1 q/k chains have relaxed
# deadlines
BLOCKS = [(0, 0), (1, 0), (2, 0), (3, 0), (0, 1), (1, 1), (2, 1), (3, 1)]
U_START = 44      # first cycle with U matmuls (chains fully drain first:
                  # chain PSUM accumulators borrow the U slots, so any
                  # overlap of the two lifetimes deadlocks the in-order PE)


def _u_cycle(w):
    """U emission cycle for wave w: 2-waves/cycle catch-up from U_START
    until U trails exp by 3 cycles.  Slower catch-up rates (1.5-1.67
    waves/cycle) measured worse: they push the last two blocks' U into
    the final cycles where they congest with the norm chains."""
    return max(w + 3, U_START + w // 2)


def _split_multi_waits(nc, max_waits=1):
    """The walrus build in this container accepts at most one sync-wait per
    instruction.  Hoist extra waits onto single-wait NOPs inserted just
    before the instruction in its engine's program order."""
    uid = [0]
    for f in nc.m.functions:
        for bb in f.blocks:
            insts = bb.instructions
            new = []
            changed = False
            for inst in insts:
                si = inst.sync_info
                if si is not None and len(si.on_wait) > max_waits:
                    waits = list(si.on_wait)
                    for w in waits[:-max_waits]:
                        nop = mybir.InstNoOp(
                            name=f"I-splitw-{uid[0]}", ins=[], outs=[])
                        uid[0] += 1
                        nop.engine = inst.engine
                        nop.sync_info = mybir.SyncInfo(
                            on_wait=[w], on_update=[])
                        new.append(nop)
                    si.on_wait = waits[-max_waits:]
                    inst.sync_info = si
                    changed = True
                new.append(inst)
            if changed:
                bb.instructions = new


def build_core_kernel() -> bass.Bass:
    nc = bass.Bass()
    xT = nc.declare_dram_parameter("xT", [DIM, N], BF16, isOutput=False)
    wqk = nc.declare_dram_parameter("wqk", [DIM, 2 * GC], BF16, isOutput=False)
    wv = nc.declare_dram_parameter("wv", [DIM, GC], BF16, isOutput=False)
    wp = nc.declare_dram_parameter("wp", [GC, DIM], BF16, isOutput=False)
    out = nc.declare_dram_parameter("out", [N, DIM], BF16, isOutput=True)

    xT_r = xT.rearrange("(kt p) n -> p kt n", p=128)
    wqk_r = wqk.rearrange("(kt p) c -> p kt c", p=128)
    wv_r = wv.rearrange("(kt p) c -> p kt c", p=128)
    wp_r = wp.rearrange("(pair p) c -> p pair c", p=128)

    with tile.TileContext(nc) as tc:
        from contextlib import ExitStack

        with ExitStack() as ctx:
            consts = ctx.enter_context(tc.tile_pool(name="consts", bufs=1))
            sbuf = ctx.enter_context(tc.tile_pool(name="sbuf", bufs=1))
            epool = ctx.enter_context(tc.tile_pool(name="epool", bufs=46))
            npool = ctx.enter_context(tc.tile_pool(name="npool", bufs=2))
            rdram = ctx.enter_context(
                tc.tile_pool(name="rdram", bufs=2, space="DRAM"))
            opool = ctx.enter_context(tc.tile_pool(name="opool", bufs=4))
            psS = ctx.enter_context(
                tc.tile_pool(name="psS", bufs=2, space="PSUM"))
            psU = ctx.enter_context(
                tc.tile_pool(name="psU", bufs=1, space="PSUM"))

            # --- resident SBUF tensors -------------------------------------
            xT_sb = sbuf.tile([128, KT, N], BF16, tag="xT")
            wqk_sb = consts.tile([128, KT, 2 * GC], BF16, tag="wqk")
            wv_sb = consts.tile([128, KT, GC], BF16, tag="wv")
            wp_sb = consts.tile([128, 2, DIM], BF16, tag="wp")
            ones_sb = consts.tile([128, 1], BF16, tag="ones")
            warm_sb = consts.tile([128, 512], BF16, tag="warm")
            qk_sb = sbuf.tile([128, 4, N], BF16, tag="qk")
            # v with a ones column appended per head ([v_h | 1], stride 65):
            # the ones column turns attention@v into a matmul that also
            # emits the softmax denominator as output row 64
            v_sb = sbuf.tile([128, NKT, HPG * 65], BF16, tag="v")
            o_sb = sbuf.tile([128, 2, N], BF16, tag="o")

            nc.vector.memset(ones_sb[:], 1.0)
            nc.vector.memset(warm_sb[:], 1.0)
            v_view = v_sb.rearrange("p nt (h c) -> p nt h c", c=65)
            nc.vector.memset(v_view[:, :, :, 64:65], 1.0)

            # --- input DMAs: whole tensors (contiguous >=512B per-partition
            # segments).  wqk first on the scalar queue: the first chains
            # need it.
            for kt in range(KT):
                nc.scalar.dma_start(out=wqk_sb[:, kt, 0:256],
                                    in_=wqk_r[:, kt, 0:256])
            nc.scalar.dma_start(out=wqk_sb[:, :, 256:512],
                                in_=wqk_r[:, :, 256:512])
            nc.scalar.dma_start(out=wv_sb[:], in_=wv_r[:])
            nc.scalar.dma_start(out=wp_sb[:], in_=wp_r[:])
            for nb in range(4):
                nc.sync.dma_start(out=xT_sb[:, :, nb * 512:(nb + 1) * 512],
                                  in_=xT_r[:, :, nb * 512:(nb + 1) * 512])

            # --- wave emitters --------------------------------------------
            def s_pair(w):
                nqb, pair = BLOCKS[w // NKT]
                nkt = w % NKT
                qt = qk_sb[:, pair, :]
                kt_sb = qk_sb[:, 2 + pair, :]
                st = psS.tile([128, 1024], F32, tag="st")
                for hh in range(2):
                    nc.tensor.matmul(
                        st[:, hh * 512:(hh + 1) * 512],
                        lhsT=kt_sb[hh * 64:(hh + 1) * 64,
                                   nkt * 128:(nkt + 1) * 128],
                        rhs=qt[hh * 64:(hh + 1) * 64,
                               nqb * 512:(nqb + 1) * 512],
                        start=True,
                        stop=True,
                    )
                return st

            def exp_wave(st):
                e_t = epool.tile([128, 1024], BF16, tag="e")
                nc.scalar.activation(
                    e_t[:], st[:],
                    mybir.ActivationFunctionType.Exp,
                    scale=SCALE,
                )
                return e_t

            def u_wave(u_pair, w, e_t):
                pair = BLOCKS[w // NKT][1]
                nkt = w % NKT
                for hh, u_t in ((0, u_pair[0]), (1, u_pair[1])):
                    h = pair * 2 + hh
                    nc.tensor.matmul(
                        u_t[0:65, :],
                        lhsT=v_sb[:, nkt, h * 65:h * 65 + 65],
                        rhs=e_t[:, hh * 512:(hh + 1) * 512],
                        start=(nkt == 0),
                        stop=(nkt == NKT - 1),
                    )

            def norm_block(b, u_pair):
                """Normalize a block's U pair into o_sb.  Blocks 0-5 first
                copy U out of PSUM (bank turnover); 6-7 go straight from
                PSUM for a shorter tail."""
                nqb, pair = BLOCKS[b]
                u_a, u_b = u_pair
                late = b >= 6
                if not late:
                    uc_a = npool.tile([65, 512], F32, tag="uc_a")
                    uc_b = npool.tile([65, 512], F32, tag="uc_b")
                    nc.vector.tensor_copy(uc_a[:], u_a[0:65, :])
                    nc.vector.tensor_copy(uc_b[:], u_b[0:65, :])
                    u_a, u_b = uc_a, uc_b
                # both denominator rows into one tile so one reciprocal
                # covers them (partition offsets must be 32-aligned and a
                # span from offset 32 may not exceed 32 partitions).  The
                # first copy fills rows 0-31 with harmless v-values so
                # every reciprocal input byte is initialized.
                dn = npool.tile([33, 512], F32, tag="dn", bufs=1)
                rec = npool.tile([33, 512], F32, tag="rec", bufs=1)
                nc.vector.tensor_copy(dn[0:32, :], u_a[32:64, :])
                nc.vector.tensor_copy(dn[32:33, :], u_a[64:65, :])
                nc.vector.tensor_copy(dn[0:1, :], u_b[64:65, :])
                nc.vector.reciprocal(rec[:], dn[:])
                rr_a = npool.tile([64, 512], F32, tag="rr_a", bufs=1)
                rr_b = npool.tile([64, 512], F32, tag="rr_b", bufs=1)
                r_dr = rdram.tile([2, 512], F32, tag="rdr")
                nc.sync.dma_start(out=r_dr[0:1, :], in_=rec[32:33, :])
                nc.sync.dma_start(out=r_dr[1:2, :], in_=rec[0:1, :])
                nc.sync.dma_start(
                    out=rr_a[:], in_=r_dr[0:1, :].to_broadcast([64, 512]))
                nc.sync.dma_start(
                    out=rr_b[:], in_=r_dr[1:2, :].to_broadcast([64, 512]))
                nc.vector.tensor_mul(
                    o_sb[0:64, pair, nqb * 512:(nqb + 1) * 512],
                    u_a[0:64, :], rr_a[:],
                )
                nc.vector.tensor_mul(
                    o_sb[64:128, pair, nqb * 512:(nqb + 1) * 512],
                    u_b[0:64, :], rr_b[:],
                )

            # --- chain FIFO -----------------------------------------------
            # Items: (deadline_cycle, pe_cost_ns, thunk).  Drained in order
            # while the PE budget lasts; anything overdue force-drains.
            fifo = deque()
            MM_NS = 215
            slot_rot = [0]

            def u_slot_tile(name):
                t = psU.tile([128, 512], F32, tag=f"s{slot_rot[0] % 4}",
                             name=name)
                slot_rot[0] += 1
                return t

            def q_chain(kind, ct_or_nt, nb, deadline, slot=None):
                state = {}

                def first_mm():
                    if slot is None:
                        state["acc"] = u_slot_tile("acc")
                    else:
                        state["acc"] = psU.tile([128, 512], F32,
                                                tag=f"s{slot}", name="acc")
                    chain_mm(0)

                def chain_mm(kt):
                    acc = state["acc"]
                    if kind == "a":
                        nc.tensor.matmul(
                            acc[:],
                            lhsT=wqk_sb[:, kt, CT_COL[ct_or_nt] * 128:
                                        CT_COL[ct_or_nt] * 128 + 128],
                            rhs=xT_sb[:, kt, nb * 512:(nb + 1) * 512],
                            start=(kt == 0), stop=(kt == KT - 1),
                        )
                    else:
                        nc.tensor.matmul(
                            acc[:, 0:GC],
                            lhsT=xT_sb[:, kt,
                                       ct_or_nt * 128:(ct_or_nt + 1) * 128],
                            rhs=wv_sb[:, kt, :],
                            start=(kt == 0), stop=(kt == KT - 1),
                        )

                def copy_out():
                    acc = state["acc"]
                    if kind == "a":
                        nc.vector.tensor_copy(
                            qk_sb[:, ct_or_nt, nb * 512:(nb + 1) * 512],
                            acc[:],
                        )
                    else:
                        nc.vector.tensor_copy(
                            v_view[:, ct_or_nt, :, 0:64],
                            acc[:, 0:GC].rearrange("p (h c) -> p h c", c=64),
                        )

                fifo.append((deadline, MM_NS, first_mm))
                for kt in range(1, KT):
                    fifo.append((deadline, MM_NS,
                                 lambda kt=kt: chain_mm(kt)))
                fifo.append((deadline, 0, copy_out))

            # chain order/deadlines: the copy must be EMITTED before the
            # first wave that consumes it (the Tile dependency tracker
            # captures deps in emission order)
            q_chain("a", 2, 2, 6)
            q_chain("a", 2, 3, 10)
            q_chain("a", 0, 1, 14)
            for nt in range(3):
                q_chain("b", nt, 0, 26 + nt)
            q_chain("a", 0, 2, 30)
            for nt in range(3, 13):
                q_chain("b", nt, 0, min(30 + nt, 42))
            q_chain("a", 0, 3, 36)
            q_chain("a", 3, 0, 37)
            q_chain("a", 1, 0, 38)
            q_chain("a", 3, 1, 39)
            q_chain("a", 3, 2, 40)
            q_chain("a", 3, 3, 41)
            q_chain("a", 1, 1, 42)
            q_chain("a", 1, 2, 43)
            q_chain("a", 1, 3, 43)
            # the last three v-chains may spill past U_START; pin them to
            # the s2/s3 slots, which U only claims at block 1 (cycle ~52)
            q_chain("b", 13, 0, 44, slot=2)
            q_chain("b", 14, 0, 45, slot=3)
            q_chain("b", 15, 0, 46, slot=2)

            # U emission map: catch-up at 2 waves/cycle after U_START until
            # trailing exp by 3 cycles
            u_emit = {}
            for w in range(NW):
                u_emit.setdefault(_u_cycle(w), []).append(w)

            # --- prologue --------------------------------------------------
            # HAM warm-up: harmless M=1 matmuls bridge the DMA wait so the
            # PE clock is at 8/8 when the first real chains run
            warm_ps = u_slot_tile("warm_ps")
            for i in range(12):
                nc.tensor.matmul(
                    warm_ps[0:1, :],
                    lhsT=ones_sb[:, 0:1], rhs=warm_sb[:],
                    start=(i == 0), stop=(i == 11),
                )
            for ct, nb in ((2, 0), (0, 0), (2, 1)):
                accp = u_slot_tile(f"accp{ct}{nb}")
                for kt in range(KT):
                    nc.tensor.matmul(
                        accp[:],
                        lhsT=wqk_sb[:, kt, CT_COL[ct] * 128:
                                    CT_COL[ct] * 128 + 128],
                        rhs=xT_sb[:, kt, nb * 512:(nb + 1) * 512],
                        start=(kt == 0), stop=(kt == KT - 1))
                nc.vector.tensor_copy(
                    qk_sb[:, ct, nb * 512:(nb + 1) * 512], accp[:])

            u_pairs = {}

            def get_u_pair(b):
                if b not in u_pairs:
                    base = (b % 2) * 2
                    t_a = psU.tile([128, 512], F32, tag=f"s{base}",
                                   name=f"ua{b}")
                    t_b = psU.tile([128, 512], F32, tag=f"s{base + 1}",
                                   name=f"ub{b}")
                    u_pairs[b] = (t_a, t_b)
                return u_pairs[b]

            # --- main loop -------------------------------------------------
            e_tiles = {}
            st_prev = s_pair(0)
            for cyc in range(NW + 6):
                budget = 1100.0
                if cyc < NW:
                    e_tiles[cyc] = exp_wave(st_prev)
                    if cyc + 1 < NW:
                        st_prev = s_pair(cyc + 1)
                        budget -= 216
                for w in u_emit.get(cyc, []):
                    b = w // NKT
                    u_wave(get_u_pair(b), w, e_tiles[w])
                    budget -= 2 * MM_NS
                    if w % NKT == NKT - 1:
                        norm_block(b, u_pairs[b])
                spent = 0.0
                while fifo and (
                    any(d is not None and d <= cyc for d, _, _ in fifo)
                    or spent + fifo[0][1] <= budget
                ):
                    _, cost, thunk = fifo.popleft()
                    thunk()
                    spent += cost
            while fifo:
                fifo.popleft()[2]()

            # --- tail: full output projection on the freed psS banks ------
            # pair-0 matmuls of the last quad flow while block 7 normalizes
            def proj_piece(mt):
                ot = opool.tile([128, DIM], BF16, tag="ot", name="ot")
                for nh in range(2):
                    acc_w = psS.tile([128, 1024], F32, tag="st",
                                     name="tailacc")
                    acc = acc_w[:, 0:512]
                    for pair in range(2):
                        nc.tensor.matmul(
                            acc[:],
                            lhsT=o_sb[:, pair, mt * 128:(mt + 1) * 128],
                            rhs=wp_sb[:, pair, nh * 512:(nh + 1) * 512],
                            start=(pair == 0), stop=(pair == 1),
                        )
                    nc.scalar.copy(ot[:, nh * 512:(nh + 1) * 512], acc[:])
                eng = nc.scalar if mt % 2 else nc.sync
                eng.dma_start(out=out[mt * 128:(mt + 1) * 128, :], in_=ot[:])

            for mt in [0, 1, 2, 3, 4, 5, 6, 7, 8, 9, 10, 11, 12, 13,
                       14, 15]:
                proj_piece(mt)

    _split_multi_waits(nc)
    return nc


_NC_CACHE = None


def _get_nc():
    global _NC_CACHE
    if _NC_CACHE is None:
        _NC_CACHE = build_core_kernel()
    return _NC_CACHE


def kernel(x, importance_weights, W_qkv, W_proj, b_proj, persistence_bias,
           _results_hook=None):
    x = np.asarray(x)
    W_qkv = np.asarray(W_qkv, dtype=np.float32)
    W_proj = np.asarray(W_proj, dtype=np.float32)
    b_proj = np.asarray(b_proj, dtype=np.float32)

    bf = ml_dtypes.bfloat16
    Q = W_qkv[:, 0:DIM]
    K = W_qkv[:, DIM:2 * DIM]
    V = W_qkv[:, 2 * DIM:3 * DIM]

    in_maps = []
    for core in range(8):
        b, g = divmod(core, 4)
        sl = slice(g * GC, (g + 1) * GC)
        in_maps.append({
            "xT": np.ascontiguousarray(x[b].T).astype(bf),
            # [k-pair0 | q-pair0 | k-pair1 | q-pair1]: the first half
            # unlocks the first chains with a half-tensor DMA
            "wqk": np.ascontiguousarray(np.concatenate(
                [K[:, sl][:, 0:128], Q[:, sl][:, 0:128],
                 K[:, sl][:, 128:256], Q[:, sl][:, 128:256]],
                axis=1)).astype(bf),
            "wv": np.ascontiguousarray(V[:, sl]).astype(bf),
            "wp": np.ascontiguousarray(W_proj[sl, :]).astype(bf),
        })

    nc = _get_nc()
    res = run_bass_kernel_spmd(nc, in_maps, list(range(8)))
    if _results_hook is not None:
        _results_hook(res)

    out = np.zeros((B, N, DIM), dtype=np.float32)
    for core in range(8):
        b = core // 4
        out[b] += res.results[core]["out"].astype(np.float32)
    out += b_proj[None, None, :]
    return out
